# revision 1
# baseline (speedup 1.0000x reference)
"""Trainium2 Bass kernel for nn_CSABlock (dual spatial-attention gating).

Reference computation:
    sa_x  = sigmoid(conv3d(concat[max_c(x), mean_c(x)], w, k=7, pad=3))
    fix_out  = move * sa_fix + fix
    move_out = fix * sa_move + move

Sharding: 8 cores = (batch 2) x (D quarters of 20 planes). Each core gets a
26-plane input slab (3-voxel halo each side) per tensor in bf16 and produces
20 output planes in bf16; the host casts f32<->bf16 and shards/gathers.

Per-core pipeline (v5):
  - Stream D in chunks (3,4,4,4,4,4,3 planes). Load tile layout:
    partition=(d,hg32), free=(c16, hp3*w96) bf16 -> 576B contiguous lines.
  - Channel max tree on DVE, sum tree split DVE/GpSimd (mean's 1/16 is
    folded into the conv weights); final tree levels write fp8e4 stats.
  - Pooled stats staged into persistent P tiles [hin_pad128, stat2, dp28,
    wp102] fp8 via per-(d,hp) SBUF->SBUF reshape DMAs.
  - Conv: 49 fp8 DoubleRow matmuls per 4-plane output group; the two
    stats ride the k-tile dim; contraction over padded H with banded
    weights lhsT[hin, (kd,kw), c, hout] scaled by WS=256.
  - Sigmoid on ScalarE with scale=1/WS (PSUM -> SBUF bf16), reshaped to
    the data layout via 4 small DMAs.
  - Gating all-bf16: fix chain on DVE, move chain mostly on GpSimd.
  - DMA rings are isolated to avoid head-of-line blocking: loads flow on
    the sync ring with nothing ahead of them; P-stage/gate/store DMAs
    (which wait on compute semaphores) share the scalar ring in
    dependency order.
"""

import sys

import numpy as np

for _p in ("/opt/trn_rl_repo",):
    if _p not in sys.path:
        sys.path.insert(0, _p)

import ml_dtypes  # noqa: E402

B, C, D, H, W = 2, 16, 80, 96, 96
KK = 7
DSLAB = 28          # slab plane indexing (plane 0 and 27 never loaded)
OUTD = 20           # output planes per core
HG, HPW = 32, 3     # h = hg*3 + hp
WPAD = 102
NPAIR = KK * KK     # 49 DoubleRow matmuls per output group
NOC = 5             # output groups of G=4 planes
G = 4
NCORES = 8
WS = 256.0          # fp8 weight scale; undone in the sigmoid
CHUNKS = [(1, 3), (4, 4), (8, 4), (12, 4), (16, 4), (20, 4), (24, 3)]
LOADD = 26          # planes 1..26 inclusive

_prog_cache: dict = {}


def _build_banded(w: np.ndarray) -> np.ndarray:
    """w: [1,2,7,7,7] f32 -> lhsT [hin_pad 128, pair 49, c 2, hout 96] fp8e4.

    out[o,h,w'] = sum_{c,kd,kw} lhsT[hq, kd*7+kw, c, h] * P[hq, c, o+1+kd, w'+kw]
    with P[h_in+3, stat, dp, w_in+3] = pooled stats (0=max, 1=sum) and
    lhsT[h+kh, kd*7+kw, c, h] = w[c,kd,kh,kw] * WS * (1/16 for c=1).
    """
    A = np.zeros((128, NPAIR, 2, 96), np.float32)
    hh = np.arange(96)
    for c in range(2):
        scale = WS if c == 0 else WS / C
        for kd in range(KK):
            for kw in range(KK):
                pair = kd * KK + kw
                for kh in range(KK):
                    A[hh + kh, pair, c, hh] = w[0, c, kd, kh, kw] * scale
    return A.astype(ml_dtypes.float8_e4m3fn)


def _build_program():
    import concourse.bass as bass
    import concourse.bacc as bacc
    import concourse.tile as tile
    from concourse import mybir
    from contextlib import ExitStack

    bf16 = mybir.dt.bfloat16
    fp8 = mybir.dt.float8e4

    nc = bacc.Bacc("TRN2")
    fxs = nc.dram_tensor("fxs", [C, LOADD, H, W], bf16, kind="ExternalInput")
    mvs = nc.dram_tensor("mvs", [C, LOADD, H, W], bf16, kind="ExternalInput")
    wgf = nc.dram_tensor("wgf", [128, NPAIR, 2, 96], fp8, kind="ExternalInput")
    wgm = nc.dram_tensor("wgm", [128, NPAIR, 2, 96], fp8, kind="ExternalInput")
    fo = nc.dram_tensor("fo", [C, OUTD, H, W], bf16, kind="ExternalOutput")
    mo = nc.dram_tensor("mo", [C, OUTD, H, W], bf16, kind="ExternalOutput")

    with tile.TileContext(nc) as tc, ExitStack() as ctx:
        singles = ctx.enter_context(tc.tile_pool(name="singles", bufs=1))
        lpf = ctx.enter_context(tc.tile_pool(name="lpf", bufs=6))
        lpm = ctx.enter_context(tc.tile_pool(name="lpm", bufs=6))
        trpool = ctx.enter_context(tc.tile_pool(name="tr", bufs=4))
        pstage = ctx.enter_context(tc.tile_pool(name="pstage", bufs=5))
        tpool = ctx.enter_context(tc.tile_pool(name="tmp", bufs=6))
        gpool = ctx.enter_context(tc.tile_pool(name="gate", bufs=2))
        gtpool = ctx.enter_context(tc.tile_pool(name="gateT", bufs=3))
        psum = ctx.enter_context(tc.tile_pool(name="psum", bufs=4, space="PSUM"))

        WGF = singles.tile([128, NPAIR, 2, 96], fp8)
        WGM = singles.tile([128, NPAIR, 2, 96], fp8)

        # Persistent pooled tensors [hin_pad, stat(max,sum), dp, wp] fp8
        PF = singles.tile([128, 2, DSLAB, WPAD], fp8)
        PM = singles.tile([128, 2, DSLAB, WPAD], fp8)
        nc.gpsimd.memset(PF[:], 0.0)
        nc.gpsimd.memset(PM[:], 0.0)

        ltiles: dict = {}

        def load_chunk(ic: int):
            p0, nd = CHUNKS[ic]
            np_ = nd * HG
            for name, dram, lpool in (("f", fxs, lpf), ("m", mvs, lpm)):
                # partition order p = d*32 + hg, free (c, hp*w): one DMA,
                # 576B contiguous lines
                L = lpool.tile([128, C, HPW * W], bf16, tag="L" + name)
                src = dram[:, p0 - 1:p0 - 1 + nd, :, :].rearrange(
                    "c d (hg hp) w -> (d hg) c (hp w)", hg=HG, hp=HPW
                )
                nc.sync.dma_start(out=L[:np_], in_=src)
                ltiles[(name, ic)] = L

        def pool_chunk(ic: int):
            p0, nd = CHUNKS[ic]
            np_ = nd * HG  # active partitions
            for name, P in (("f", PF), ("m", PM)):
                L = ltiles[(name, ic)]

                # channel-reduction trees -> PS [np_, stat2, hp3, 96] fp8
                PS = pstage.tile([128, 2, HPW, W], fp8, tag="PS")
                Tmax = trpool.tile([128, 8, HPW * W], bf16, tag="Tmax")
                Tsum = trpool.tile([128, 8, HPW * W], bf16, tag="Tsum")
                Lv = L[:np_]
                PSv = PS[:np_].rearrange("p s hp w -> p s (hp w)")
                # max tree on DVE; sum tree L1 on DVE, rest on GpSimd
                nc.vector.tensor_max(Tmax[:np_], Lv[:, 0:8, :], Lv[:, 8:16, :])
                nc.vector.tensor_add(Tsum[:np_], Lv[:, 0:8, :], Lv[:, 8:16, :])
                nc.vector.tensor_max(Tmax[:np_, 0:4], Tmax[:np_, 0:4], Tmax[:np_, 4:8])
                nc.gpsimd.tensor_add(Tsum[:np_, 0:4], Tsum[:np_, 0:4], Tsum[:np_, 4:8])
                nc.vector.tensor_max(Tmax[:np_, 0:2], Tmax[:np_, 0:2], Tmax[:np_, 2:4])
                nc.gpsimd.tensor_add(Tsum[:np_, 0:2], Tsum[:np_, 0:2], Tsum[:np_, 2:4])
                nc.vector.tensor_max(PSv[:, 0], Tmax[:np_, 0], Tmax[:np_, 1])
                nc.gpsimd.tensor_add(PSv[:, 1], Tsum[:np_, 0], Tsum[:np_, 1])

                # stage into P: per (d, hp): src partitions d*32..d*32+31
                # (hg), free (stat, w); dst partitions 3+hp+3*hg (step 3).
                # Early chunks go fully on the scalar ring (the sync ring is
                # still draining the upfront loads, which would delay them);
                # later chunks alternate across both rings.
                for d in range(nd):
                    for hp in range(HPW):
                        if ic < 3:
                            eng = nc.scalar
                        else:
                            eng = nc.scalar if (d + hp) % 2 else nc.sync
                        eng.dma_start(
                            out=P[3 + hp:3 + hp + 94:3, :, p0 + d, 3:3 + W],
                            in_=PS[d * HG:(d + 1) * HG, :, hp, :],
                        )

        def conv_group(oc: int):
            o0 = G * oc
            gates = {}
            for name, P, WG in (("f", PF, WGF), ("m", PM, WGM)):
                acc = psum.tile([96, G, 96], mybir.dt.float32, tag="acc")
                for kd in range(KK):
                    dsl = slice(o0 + 1 + kd, o0 + 1 + kd + G)
                    for kw in range(KK):
                        nc.tensor.matmul(
                            acc[:],
                            WG[:, kd * KK + kw],
                            P[:, 0:2, dsl, kw:kw + 96],
                            start=(kd == 0 and kw == 0),
                            stop=(kd == KK - 1 and kw == KK - 1),
                            perf_mode=mybir.MatmulPerfMode.DoubleRow,
                        )
                gate = gpool.tile([96, G, 96], bf16, tag="gate")
                nc.scalar.activation(
                    out=gate[:], in_=acc[:],
                    func=mybir.ActivationFunctionType.Sigmoid,
                    scale=1.0 / WS,
                )
                # [96=h, (d,w)] -> gateT [128=(hg,d), (hp,w)]
                gateT = gtpool.tile([128, HPW, W], bf16, tag="gT")
                for d in range(G):
                    nc.scalar.dma_start(
                        out=gateT[d * HG:(d + 1) * HG, :, :], in_=gate[:, d, :]
                    )
                gates[name] = gateT
            return gates

        def elementwise(oc: int, gates):
            ic = oc + 1
            Lf, Lm = ltiles[("f", ic)], ltiles[("m", ic)]
            gf = (
                gates["f"][:].rearrange("p hp w -> p (hp w)").unsqueeze(1)
                .broadcast_to((128, 8, HPW * W))
            )
            gm = (
                gates["m"][:].rearrange("p hp w -> p (hp w)").unsqueeze(1)
                .broadcast_to((128, 8, HPW * W))
            )
            for q in range(2):
                cs = slice(q * 8, (q + 1) * 8)
                Tf = tpool.tile([128, 8, HPW * W], bf16, tag="T")
                Tm = tpool.tile([128, 8, HPW * W], bf16, tag="T")
                # fix chain on DVE: fo = move*gf + fix (broadcast operand
                # goes in slot 0; slot-1 broadcasts measured ~2x slower)
                nc.vector.tensor_mul(Tf[:], gf, Lm[:, cs, :])
                nc.vector.tensor_add(Tf[:], Tf[:], Lf[:, cs, :])
                # move chain on GpSimd: mo = fix*gm + move
                nc.gpsimd.tensor_mul(Tm[:], gm, Lf[:, cs, :])
                if q == 0:
                    nc.vector.tensor_add(Tm[:], Tm[:], Lm[:, cs, :])
                else:
                    nc.gpsimd.tensor_add(Tm[:], Tm[:], Lm[:, cs, :])
                # stores ride the sync ring: all loads were issued up front,
                # so nothing queues behind these waits
                for T, dram in ((Tf, fo), (Tm, mo)):
                    dst = dram[cs, G * oc:G * oc + G, :, :].rearrange(
                        "c d (hg hp) w -> (d hg) c (hp w)", hg=HG, hp=HPW
                    )
                    nc.sync.dma_start(out=dst, in_=T[:])

        # software pipeline: all loads issued up front (L pools are deep
        # enough), keeping the sync ring free of compute-dependent waits.
        # Trees for the last two chunks are deferred into the oc loop so the
        # in-order DVE/GpSimd queues interleave them with early gating
        # instead of front-loading all trees and tail-loading all gating.
        for ic in range(6):
            load_chunk(ic)
            if ic == 2:
                # weight loads ride behind the first chunks: they head the
                # sync ring otherwise, delaying the first trees ~9us, but
                # aren't needed until conv(0)
                nc.sync.dma_start(out=WGF[:], in_=wgf[:])
                nc.sync.dma_start(out=WGM[:], in_=wgm[:])
        for ic in range(5):
            pool_chunk(ic)
        for oc in range(NOC):
            if oc == 0:
                load_chunk(6)
            gates = conv_group(oc)
            elementwise(oc, gates)
            if oc == 0:
                pool_chunk(5)
            elif oc == 1:
                pool_chunk(6)

    nc.compile()
    return nc


def _get_program():
    if "nc" not in _prog_cache:
        _prog_cache["nc"] = _build_program()
    return _prog_cache["nc"]


def _shard(fix, move, Af, Am):
    in_maps = []
    for core in range(NCORES):
        b, dq = core // 4, core % 4
        lo = 20 * dq - 3  # global index of slab plane 1
        s0, s1 = max(lo, 0), min(lo + LOADD, D)
        slab_f = np.zeros((C, LOADD, H, W), ml_dtypes.bfloat16)
        slab_m = np.zeros((C, LOADD, H, W), ml_dtypes.bfloat16)
        slab_f[:, s0 - lo:s1 - lo] = fix[b, :, s0:s1]
        slab_m[:, s0 - lo:s1 - lo] = move[b, :, s0:s1]
        in_maps.append({"fxs": slab_f, "mvs": slab_m, "wgf": Af, "wgm": Am})
    return in_maps


def kernel(fix, move, w_f2m, w_m2f, __trace=False):
    fix = np.asarray(fix, dtype=np.float32).astype(ml_dtypes.bfloat16)
    move = np.asarray(move, dtype=np.float32).astype(ml_dtypes.bfloat16)
    Af = _build_banded(np.asarray(w_f2m, dtype=np.float32))
    Am = _build_banded(np.asarray(w_m2f, dtype=np.float32))

    nc = _get_program()
    in_maps = _shard(fix, move, Af, Am)

    from concourse.bass_utils import run_bass_kernel_spmd

    res = run_bass_kernel_spmd(
        nc, in_maps, core_ids=list(range(NCORES)), trace=__trace
    )
    _prog_cache["last_results"] = res

    fix_out = np.empty((B, C, D, H, W), np.float32)
    move_out = np.empty((B, C, D, H, W), np.float32)
    for core in range(NCORES):
        b, dq = core // 4, core % 4
        fix_out[b, :, 20 * dq:20 * dq + 20] = res.results[core]["fo"].astype(
            np.float32
        )
        move_out[b, :, 20 * dq:20 * dq + 20] = res.results[core]["mo"].astype(
            np.float32
        )
    return fix_out, move_out



# revision 9
# speedup vs baseline: 1.3702x; 1.3702x over previous
"""Trainium2 Bass kernel for nn_CSABlock (dual spatial-attention gating).

Reference computation:
    sa_x  = sigmoid(conv3d(concat[max_c(x), mean_c(x)], w, k=7, pad=3))
    fix_out  = move * sa_fix + fix
    move_out = fix * sa_move + move

Sharding: 8 cores = (batch 2) x (D quarters of 20 planes). Each core gets a
26-plane input slab (3-voxel halo each side) per tensor in bf16 and produces
20 output planes in bf16; the host casts f32<->bf16 and shards/gathers.

v6 design notes (evidence from the v5 trace):
  - v5 ran DVE and GpSimd tensor_tensor streams concurrently. They arbitrate
    an exclusive SBUF shared-port lock, so they serialize: DVE ops overlapped
    by GpSimd measured 3.0x their cost-model time vs 1.07x when alone.
    v6 therefore runs ALL elementwise work on DVE (2 elem/cyc bf16) and
    leaves GpSimd idle; ACT only does memzero + sigmoid.
  - fix/move ride one extra tensor dim everywhere (loads, trees, stats,
    gates, gating, stores), halving instruction and DMA counts.
  - All DMA is HWDGE. sync queue: loads, P-stage, stores (issue order
    matches semaphore resolution order, so no head-of-line stalls).
    scalar queue: weights + gate reshapes (right after their sigmoids).
  - Dummy matmuls on the weight tile bridge the PE-idle gap before the
    first conv group so HAM doesn't drop the PE clock to 1.2 GHz.

Per-core pipeline:
  - Stream D in chunks (3,4,4,4,4,4,3 planes). Load tile layout:
    partition=(d,hg32), free=(t2, c16, hp3*w96) bf16 -> 576B lines.
  - Channel max/sum trees on DVE (mean's 1/16 folded into conv weights);
    final tree level writes fp8 stats.
  - Stats staged into persistent P [hin_pad128, t2, stat2, dp28, wp102]
    fp8 via per-d SBUF->SBUF reshape DMAs.
  - Conv: 49 fp8 DoubleRow matmuls per 4-plane output group per tensor;
    stats ride the k-tile dim; contraction over padded H with banded
    weights lhsT[hin, t, (kd,kw), c, hout] scaled by WS=256.
  - Sigmoid on ACT with scale=1/WS (PSUM -> SBUF bf16), reshaped to the
    data layout via 4 small DMAs per group.
  - Gating all-bf16 on DVE, one 16-channel op per (tensor, mul/add).
"""

import sys

import numpy as np

for _p in ("/opt/trn_rl_repo",):
    if _p not in sys.path:
        sys.path.insert(0, _p)

import ml_dtypes  # noqa: E402

B, C, D, H, W = 2, 16, 80, 96, 96
KK = 7
DSLAB = 28          # slab plane indexing (plane 0 and 27 never loaded)
OUTD = 20           # output planes per core
HG, HPW = 32, 3     # h = hg*3 + hp
WPAD = 104          # 102 needed; padded to /4 so ACT memzero can bitcast u32
NPAIR = KK * KK     # 49 DoubleRow matmuls per output group
NOC = 5             # output groups of G=4 planes
G = 4
NCORES = 8
WS = 256.0          # fp8 weight scale; undone in the sigmoid
CHUNKS = [(1, 3), (4, 4), (8, 4), (12, 4), (16, 4), (20, 4), (24, 3)]
LOADD = 26          # planes 1..26 inclusive
NWARM = 32          # PE warm-up dummy matmuls before conv group 0

_prog_cache: dict = {}


def _build_banded(w: np.ndarray) -> np.ndarray:
    """w: [1,2,7,7,7] f32 -> lhsT [hin_pad 128, pair 49, c 2, hout 96] f32.

    out[o,h,w'] = sum_{c,kd,kw} lhsT[hq, kd*7+kw, c, h] * P[hq, c, o+1+kd, w'+kw]
    with P[h_in+3, stat, dp, w_in+3] = pooled stats (0=max, 1=sum) and
    lhsT[h+kh, kd*7+kw, c, h] = w[c,kd,kh,kw] * WS * (1/16 for c=1).
    """
    A = np.zeros((128, NPAIR, 2, 96), np.float32)
    hh = np.arange(96)
    for c in range(2):
        scale = WS if c == 0 else WS / C
        for kd in range(KK):
            for kw in range(KK):
                pair = kd * KK + kw
                for kh in range(KK):
                    A[hh + kh, pair, c, hh] = w[0, c, kd, kh, kw] * scale
    return A


def _build_program():
    import concourse.bass as bass  # noqa: F401
    import concourse.bacc as bacc
    import concourse.tile as tile
    from concourse import mybir
    from contextlib import ExitStack

    bf16 = mybir.dt.bfloat16
    fp8 = mybir.dt.float8e4

    nc = bacc.Bacc("TRN2")
    xin = nc.dram_tensor("xin", [2, C, LOADD, H, W], bf16, kind="ExternalInput")
    wgt = nc.dram_tensor("wgt", [128, 2, NPAIR, 2, 96], fp8, kind="ExternalInput")
    xout = nc.dram_tensor("xout", [2, C, OUTD, H, W], bf16, kind="ExternalOutput")

    with tile.TileContext(nc) as tc, ExitStack() as ctx:
        singles = ctx.enter_context(tc.tile_pool(name="singles", bufs=1))
        lp = ctx.enter_context(tc.tile_pool(name="lp", bufs=5))
        trpool = ctx.enter_context(tc.tile_pool(name="tr", bufs=2))
        pspool = ctx.enter_context(tc.tile_pool(name="pstage", bufs=2))
        tpool = ctx.enter_context(tc.tile_pool(name="tmp", bufs=2))
        gpool = ctx.enter_context(tc.tile_pool(name="gate", bufs=2))
        gtpool = ctx.enter_context(tc.tile_pool(name="gateT", bufs=2))
        psum = ctx.enter_context(tc.tile_pool(name="psum", bufs=4, space="PSUM"))
        psumw = ctx.enter_context(tc.tile_pool(name="psumw", bufs=1, space="PSUM"))

        WG = singles.tile([128, 2, NPAIR, 2, 96], fp8)
        # Persistent pooled stats [hin_pad, tensor, stat(max,sum), dp, wp]
        P = singles.tile([128, 2, 2, DSLAB, WPAD], fp8)

        # ACT zeroes P (cheap, ACT is idle early; GpSimd memset would
        # grab the DVE-shared SBUF port). Weights load right behind it
        # on the scalar ring so the sync ring drains pure input loads.
        nc.scalar.memzero(P[:].rearrange("p t s d w -> p t s (d w)"))
        nc.scalar.dma_start(out=WG[:], in_=wgt[:])

        ltiles: dict = {}

        def load_chunk(ic: int):
            p0, nd = CHUNKS[ic]
            np_ = nd * HG
            L = lp.tile([128, 2, C, HPW * W], bf16, tag="L")
            src = xin[:, :, p0 - 1:p0 - 1 + nd, :, :].rearrange(
                "t c d (hg hp) w -> (d hg) t c (hp w)", hg=HG, hp=HPW
            )
            nc.sync.dma_start(out=L[:np_], in_=src)
            ltiles[ic] = L

        def trees(ic: int):
            """Channel max+sum trees for both tensors on DVE, then stage
            the fp8 stats into P with one reshape DMA per plane."""
            p0, nd = CHUNKS[ic]
            np_ = nd * HG
            Lv = ltiles[ic][:np_]
            Tmax = trpool.tile([128, 2, 8, HPW * W], bf16, tag="Tmax")
            Tsum = trpool.tile([128, 2, 8, HPW * W], bf16, tag="Tsum")
            PS = pspool.tile([128, 2, 2, HPW, W], fp8, tag="PS")
            v = nc.vector
            v.tensor_max(Tmax[:np_], Lv[:, :, 0:8, :], Lv[:, :, 8:16, :])
            v.tensor_add(Tsum[:np_], Lv[:, :, 0:8, :], Lv[:, :, 8:16, :])
            v.tensor_max(Tmax[:np_, :, 0:4], Tmax[:np_, :, 0:4], Tmax[:np_, :, 4:8])
            v.tensor_add(Tsum[:np_, :, 0:4], Tsum[:np_, :, 0:4], Tsum[:np_, :, 4:8])
            v.tensor_max(Tmax[:np_, :, 0:2], Tmax[:np_, :, 0:2], Tmax[:np_, :, 2:4])
            v.tensor_add(Tsum[:np_, :, 0:2], Tsum[:np_, :, 0:2], Tsum[:np_, :, 2:4])
            PSm = PS[:np_, :, 0].rearrange("p t hp w -> p t (hp w)")
            PSs = PS[:np_, :, 1].rearrange("p t hp w -> p t (hp w)")
            v.tensor_max(PSm, Tmax[:np_, :, 0], Tmax[:np_, :, 1])
            v.tensor_add(PSs, Tsum[:np_, :, 0], Tsum[:np_, :, 1])
            return PS

        def pstage(ic: int, PS):
            # stage into P: per (plane, hp): src partitions d*32+hg, free
            # (t,s,w); dst partitions 3+hp+3*hg (step 3). Both sides merge
            # (t,s) so the DMA balancer sees 3 dims.
            p0, nd = CHUNKS[ic]
            for d in range(nd):
                for hp in range(HPW):
                    nc.sync.dma_start(
                        out=P[3 + hp:3 + hp + 94:3, :, :, p0 + d, 3:3 + W],
                        in_=PS[d * HG:(d + 1) * HG, :, :, hp, :],
                    )

        def warm_pe():
            # Garbage matmuls on the (read-only) weight tile keep the PE
            # active before conv group 0 so HAM holds the 2.4 GHz clock.
            acc = psumw.tile([96, G, 96], mybir.dt.float32, tag="warm")
            lhs = WG[:, 0, 0, 0, :]
            rhs = WG[:, 1, 0:G, 0, :]
            for _ in range(NWARM):
                nc.tensor.matmul(acc[:], lhs, rhs, start=True, stop=True)

        def conv_group(oc: int):
            o0 = G * oc
            gate = gpool.tile([96, 2, G, 96], bf16, tag="gate")
            for t in range(2):
                acc = psum.tile([96, G, 96], mybir.dt.float32, tag="acc")
                for kd in range(KK):
                    dsl = slice(o0 + 1 + kd, o0 + 1 + kd + G)
                    for kw in range(KK):
                        nc.tensor.matmul(
                            acc[:],
                            WG[:, t, kd * KK + kw],
                            P[:, t, 0:2, dsl, kw:kw + 96],
                            start=(kd == 0 and kw == 0),
                            stop=(kd == KK - 1 and kw == KK - 1),
                            perf_mode=mybir.MatmulPerfMode.DoubleRow,
                        )
                nc.scalar.activation(
                    out=gate[:, t], in_=acc[:],
                    func=mybir.ActivationFunctionType.Sigmoid,
                    scale=1.0 / WS,
                )
            # [96=h, (t,d,w)] -> gateT [128=(d,hg), t, (hp,w)]
            gateT = gtpool.tile([128, 2, HPW, W], bf16, tag="gT")
            for d in range(G):
                for t in range(2):
                    nc.scalar.dma_start(
                        out=gateT[d * HG:(d + 1) * HG, t],
                        in_=gate[:, t, d, :],
                    )
            return gateT

        def gating(oc: int, gateT):
            L = ltiles[oc + 1]
            gf = (
                gateT[:, 0].rearrange("p hp w -> p (hp w)").unsqueeze(1)
                .broadcast_to((128, C, HPW * W))
            )
            gm = (
                gateT[:, 1].rearrange("p hp w -> p (hp w)").unsqueeze(1)
                .broadcast_to((128, C, HPW * W))
            )
            T = tpool.tile([128, 2, C, HPW * W], bf16, tag="T")
            v = nc.vector
            # broadcast operand in slot 0 (slot-1 broadcast is ~2x slower)
            v.tensor_mul(T[:, 0], gf, L[:, 1])
            v.tensor_add(T[:, 0], T[:, 0], L[:, 0])
            v.tensor_mul(T[:, 1], gm, L[:, 0])
            v.tensor_add(T[:, 1], T[:, 1], L[:, 1])
            dst = xout[:, :, G * oc:G * oc + G, :, :].rearrange(
                "t c d (hg hp) w -> (d hg) t c (hp w)", hg=HG, hp=HPW
            )
            nc.sync.dma_start(out=dst, in_=T[:])

        # Software pipeline. DVE queue order: t0..t4, g0, t5, g1, t6,
        # g2, g3, g4 (trees stay ahead of gating so the cross-engine
        # gate wait never head-of-line blocks tree work). Sync queue
        # issue order is monotone in semaphore-resolution time:
        #   L0-3, P0[t0], P1[t1], L4, P2[t2], L5[t0/buf], P3[t3],
        #   P4[t4], L6[g0/buf], S0[g0], P5[t5], S1[g1], P6[t6], S2-4.
        load_chunk(0)
        load_chunk(1)
        load_chunk(2)
        load_chunk(3)
        ps0 = trees(0)
        ps1 = trees(1)
        pstage(0, ps0)
        pstage(1, ps1)
        load_chunk(4)
        ps2 = trees(2)
        pstage(2, ps2)
        load_chunk(5)
        ps3 = trees(3)
        pstage(3, ps3)
        warm_pe()
        gts = {0: conv_group(0)}
        ps4 = trees(4)
        pstage(4, ps4)
        gating(0, gts[0])
        load_chunk(6)  # reuses chunk 1's buffer; gating(0) read it above
        gts[1] = conv_group(1)
        ps5 = trees(5)
        pstage(5, ps5)
        gating(1, gts[1])
        gts[2] = conv_group(2)
        ps6 = trees(6)
        pstage(6, ps6)
        gating(2, gts[2])
        gts[3] = conv_group(3)
        gating(3, gts[3])
        gts[4] = conv_group(4)
        gating(4, gts[4])

    nc.compile()
    return nc


def _get_program():
    if "nc" not in _prog_cache:
        _prog_cache["nc"] = _build_program()
    return _prog_cache["nc"]


def _shard(fix, move, Wm):
    in_maps = []
    for core in range(NCORES):
        b, dq = core // 4, core % 4
        lo = 20 * dq - 3  # global index of slab plane 1
        s0, s1 = max(lo, 0), min(lo + LOADD, D)
        slab = np.zeros((2, C, LOADD, H, W), ml_dtypes.bfloat16)
        slab[0, :, s0 - lo:s1 - lo] = fix[b, :, s0:s1]
        slab[1, :, s0 - lo:s1 - lo] = move[b, :, s0:s1]
        in_maps.append({"xin": slab, "wgt": Wm})
    return in_maps


def kernel(fix, move, w_f2m, w_m2f, __trace=False):
    fix = np.asarray(fix, dtype=np.float32).astype(ml_dtypes.bfloat16)
    move = np.asarray(move, dtype=np.float32).astype(ml_dtypes.bfloat16)
    Af = _build_banded(np.asarray(w_f2m, dtype=np.float32))
    Am = _build_banded(np.asarray(w_m2f, dtype=np.float32))
    Wm = np.ascontiguousarray(
        np.stack([Af, Am]).transpose(1, 0, 2, 3, 4)
    ).astype(ml_dtypes.float8_e4m3fn)

    nc = _get_program()
    in_maps = _shard(fix, move, Wm)

    from concourse.bass_utils import run_bass_kernel_spmd

    res = run_bass_kernel_spmd(
        nc, in_maps, core_ids=list(range(NCORES)), trace=__trace
    )
    _prog_cache["last_results"] = res

    fix_out = np.empty((B, C, D, H, W), np.float32)
    move_out = np.empty((B, C, D, H, W), np.float32)
    for core in range(NCORES):
        b, dq = core // 4, core % 4
        out = res.results[core]["xout"].astype(np.float32)
        fix_out[b, :, 20 * dq:20 * dq + 20] = out[0]
        move_out[b, :, 20 * dq:20 * dq + 20] = out[1]
    return fix_out, move_out
